# revision 4
# baseline (speedup 1.0000x reference)
"""Trainium2 Bass kernel for nn_Net_12902081757308 (moe_routing).

Mixture-of-expert-kernels 3D conv + InstanceNorm + Mish.

Math: gate g = softmax_E(t @ gate_w.T + gate_b) [N,E,CO]; per-sample mixed
5^3 kernel w[n] = sum_e g[n,e,co] * expert_e[co,ci,kd,kh,kw]; y = conv3d(x, w)
SAME; InstanceNorm3d (biased var, eps=1e-5, affine gamma/beta); Mish.

Sharding (8 cores): core c -> (sample n=c//2, depth-half dh=c%2). Each core
computes all CO=64 channels for 24 of 48 output depth planes. InstanceNorm
stats are reduced across the core pairs with a tiny AllReduce.

Device algorithm per core:
  Pass B (conv): contraction (CI*5^3=4000) split into 35 matmul chunks:
    25 chunks (kh,kw): K=128 rows (kd 0..3 x ci) against a depth-replicated
      SBUF buffer A (partition block j = padded plane d+j),
    5 chunks (kw): K=128 rows (kh 0..3 x ci) at kd=4 against an h-shifted
      buffer C (partition block j = plane d+4 shifted j rows),
    5 leftover chunks (kw): K=32 rows (ci) at kd=4,kh=4 from C block 0.
  All 35 accumulate into PSUM [64co, 8h, 48w] in f32 (inputs f32r).
  ACT Copy/Square with accum_out collect per-channel sum/sumsq.
  Pass C: AllReduce partial stats over core pairs; build affine s,b with
    s=gamma*rstd, b=beta-mu*s.
  Pass D: mish(z)=z*(q-1)/(q+1), q=(1+e^z)^2, z=y*s+b via ACT Exp/Square
    + DVE rational, plane-pairs packed to 128 partitions.
"""
import numpy as np
from contextlib import ExitStack

E, CI, CO, K, T = 5, 32, 64, 5, 3
N, D = 4, 48
PD = D + 4            # padded spatial extent (52)
PLANE = PD * PD       # 2704
TD = D // 2           # output depths per core (24)
NPLANES = TD + 5      # input planes staged per core (28 + 1 guard)
VOL = D * D * D       # 110592 elements per (n, co) instance
OUTP = D * D          # 2304 per output plane
EPS = 1e-5
NCORES = 8
NCHUNK = 35
HTILES = (0, 8, 16, 24, 32, 40)
NTILES = TD * len(HTILES)  # 144

_CACHE = {}


def _build_nc():
    from concourse import bacc, mybir, tile

    dt = mybir.dt
    AFT = mybir.ActivationFunctionType

    nc = bacc.Bacc("TRN2", target_bir_lowering=False, debug=False,
                   num_devices=NCORES)
    xp_ap = nc.dram_tensor("xp", [CI, NPLANES * PLANE], dt.float32r,
                           kind="ExternalInput").ap()
    wl_ap = nc.dram_tensor("wl", [128, NCHUNK * CO], dt.float32r,
                           kind="ExternalInput").ap()
    gb_ap = nc.dram_tensor("gb", [CO, 2], dt.float32,
                           kind="ExternalInput").ap()
    out_ap = nc.dram_tensor("out", [CO, TD * OUTP], dt.float32,
                            kind="ExternalOutput").ap()

    with tile.TileContext(nc) as tc, ExitStack() as ctx:
        cpool = ctx.enter_context(tc.tile_pool(name="const", bufs=1))
        apool = ctx.enter_context(tc.tile_pool(name="abuf", bufs=2))
        cbpool = ctx.enter_context(tc.tile_pool(name="cbuf", bufs=2))
        ppool = ctx.enter_context(tc.tile_pool(name="psum", bufs=6,
                                               space="PSUM"))
        ypool = ctx.enter_context(tc.tile_pool(name="ysb", bufs=4))
        spool = ctx.enter_context(tc.tile_pool(name="stats", bufs=1))
        dpool = ctx.enter_context(tc.tile_pool(name="mish", bufs=2))
        drampool = ctx.enter_context(tc.tile_pool(name="dram", bufs=1,
                                                  space="DRAM"))

        wsb = cpool.tile([128, NCHUNK * CO], dt.float32r)
        nc.sync.dma_start(wsb[:], wl_ap[:])
        gbt = cpool.tile([CO, 2], dt.float32)
        nc.sync.dma_start(gbt[:], gb_ap[:])

        sums = spool.tile([CO, NTILES], dt.float32)
        sumsq = spool.tile([CO, NTILES], dt.float32)
        ydram = drampool.tile([CO, TD * OUTP], dt.float32)

        # ---- Pass B: conv + partial stats ----
        # Matmul free dim = contiguous 416-elem span (8 padded 52-rows) of
        # the flat plane; pad columns (w>=48) become garbage PSUM columns
        # that the extraction APs below skip. (Strided rhs halves PE rate.)
        SPAN = 8 * PD  # 416
        ti = 0
        for d in range(TD):
            A = apool.tile([128, PLANE + 16], dt.float32r)
            for j in range(4):
                nc.sync.dma_start(
                    A[32 * j:32 * j + 32, 0:PLANE],
                    xp_ap[:, (d + j) * PLANE:(d + j + 1) * PLANE])
            Ct = cbpool.tile([128, PLANE + 16], dt.float32r)
            for j in range(4):
                off = (d + 4) * PLANE + j * PD
                nc.sync.dma_start(Ct[32 * j:32 * j + 32, 0:PLANE],
                                  xp_ap[:, off:off + PLANE])
            for h0 in HTILES:
                p0 = h0 * PD
                ps = ppool.tile([CO, 8, PD], dt.float32)
                psf = ps[:].rearrange("p a b -> p (a b)")
                c = 0
                for kh in range(K):
                    for kw in range(K):
                        off = p0 + kh * PD + kw
                        nc.tensor.matmul(
                            psf, wsb[:, c * CO:(c + 1) * CO],
                            A[:, off:off + SPAN],
                            start=(c == 0), stop=False)
                        c += 1
                for kw in range(K):
                    nc.tensor.matmul(
                        psf, wsb[:, c * CO:(c + 1) * CO],
                        Ct[:, p0 + kw:p0 + kw + SPAN],
                        start=False, stop=False)
                    c += 1
                for kw in range(K):
                    off = p0 + 4 * PD + kw
                    nc.tensor.matmul(
                        psf, wsb[0:32, c * CO:(c + 1) * CO],
                        Ct[0:32, off:off + SPAN],
                        start=False, stop=(c == NCHUNK - 1))
                    c += 1
                ysb = ypool.tile([CO, 8, 48], dt.float32)
                nc.scalar.activation(ysb[:], ps[:, :, 0:48], AFT.Copy,
                                     accum_out=sums[:, ti:ti + 1])
                sqsb = ypool.tile([CO, 8, 48], dt.float32)
                nc.scalar.activation(sqsb[:], ps[:, :, 0:48], AFT.Square,
                                     accum_out=sumsq[:, ti:ti + 1])
                nc.sync.dma_start(
                    ydram[:, d * OUTP + h0 * 48:d * OUTP + h0 * 48 + 384],
                    ysb[:].rearrange("p a b -> p (a b)"))
                ti += 1

        # ---- Pass C: stats reduce + AllReduce + affine ----
        ccin = spool.tile([CO, 2], dt.float32)
        nc.vector.reduce_sum(ccin[:, 0:1], sums[:], axis=mybir.AxisListType.X)
        nc.vector.reduce_sum(ccin[:, 1:2], sumsq[:], axis=mybir.AxisListType.X)
        cin_d = drampool.tile([CO, 2], dt.float32)
        cout_d = drampool.tile([CO, 2], dt.float32)
        nc.sync.dma_start(cin_d[:], ccin[:])
        nc.gpsimd.collective_compute(
            "AllReduce", mybir.AluOpType.add,
            replica_groups=[[0, 1], [2, 3], [4, 5], [6, 7]],
            ins=[cin_d.opt()], outs=[cout_d.opt()])
        st = spool.tile([CO, 2], dt.float32)
        nc.sync.dma_start(st[:], cout_d[:])

        mu = spool.tile([CO, 1], dt.float32)
        nc.vector.tensor_scalar_mul(mu[:], st[:, 0:1], 1.0 / VOL)
        m2 = spool.tile([CO, 1], dt.float32)
        nc.vector.tensor_scalar_mul(m2[:], st[:, 1:2], 1.0 / VOL)
        musq = spool.tile([CO, 1], dt.float32)
        nc.vector.tensor_mul(musq[:], mu[:], mu[:])
        var = spool.tile([CO, 1], dt.float32)
        nc.vector.tensor_scalar(var[:], m2[:], musq[:, 0:1], EPS,
                                mybir.AluOpType.subtract, mybir.AluOpType.add)
        std = spool.tile([CO, 1], dt.float32)
        nc.scalar.activation(std[:], var[:], AFT.Sqrt)
        rstd = spool.tile([CO, 1], dt.float32)
        nc.vector.reciprocal(rstd[:], std[:])
        sv = spool.tile([CO, 1], dt.float32)
        nc.vector.tensor_mul(sv[:], rstd[:], gbt[:, 0:1])
        mus = spool.tile([CO, 1], dt.float32)
        nc.vector.tensor_mul(mus[:], mu[:], sv[:])
        bv = spool.tile([CO, 1], dt.float32)
        nc.vector.tensor_sub(bv[:], gbt[:, 1:2], mus[:])
        # duplicate across both partition halves for 128-wide pass D
        sb2 = spool.tile([128, 2], dt.float32)
        nc.sync.dma_start(sb2[0:CO, 0:1], sv[:])
        nc.sync.dma_start(sb2[CO:128, 0:1], sv[:])
        nc.sync.dma_start(sb2[0:CO, 1:2], bv[:])
        nc.sync.dma_start(sb2[CO:128, 1:2], bv[:])

        # ---- Pass D: z=y*s+b; mish(z) = z*(q-1)/(q+1), q=(1+e^z)^2 ----
        for dd in range(TD // 2):
            yt = dpool.tile([128, OUTP], dt.float32)
            nc.sync.dma_start(yt[0:CO, :],
                              ydram[:, (2 * dd) * OUTP:(2 * dd + 1) * OUTP])
            nc.sync.dma_start(yt[CO:128, :],
                              ydram[:, (2 * dd + 1) * OUTP:(2 * dd + 2) * OUTP])
            e = dpool.tile([128, OUTP], dt.float32)
            nc.scalar.activation(e[:], yt[:], AFT.Exp,
                                 scale=sb2[:, 0:1], bias=sb2[:, 1:2])
            nc.scalar.activation(e[:], e[:], AFT.Square, bias=1.0)
            z = dpool.tile([128, OUTP], dt.float32)
            nc.vector.tensor_scalar(z[:], yt[:], sb2[:, 0:1], sb2[:, 1:2],
                                    mybir.AluOpType.mult, mybir.AluOpType.add)
            r = dpool.tile([128, OUTP], dt.float32)
            nc.vector.tensor_scalar_add(r[:], e[:], 1.0)
            nc.vector.reciprocal_approx_fast(r[:], r[:])
            nc.vector.scalar_tensor_tensor(z[:], e[:], -1.0, z[:],
                                           mybir.AluOpType.add,
                                           mybir.AluOpType.mult)
            nc.vector.tensor_mul(z[:], z[:], r[:])
            nc.sync.dma_start(out_ap[:, (2 * dd) * OUTP:(2 * dd + 1) * OUTP],
                              z[0:CO, :])
            nc.sync.dma_start(out_ap[:, (2 * dd + 1) * OUTP:(2 * dd + 2) * OUTP],
                              z[CO:128, :])
    nc.compile()
    return nc


def _host_prep(x, t, w5, w3, w1, wa3, wa5, gate_w, gate_b, gamma, beta):
    f32 = np.float32
    x = np.asarray(x, f32)
    t = np.asarray(t, f32)
    logits = t @ np.asarray(gate_w, f32).T + np.asarray(gate_b, f32)
    lg = logits.reshape(N, E, CO)
    lg = lg - lg.max(axis=1, keepdims=True)
    eg = np.exp(lg)
    g = eg / eg.sum(axis=1, keepdims=True)          # [N, E, CO]

    def pad_k(w, p):
        return np.pad(np.asarray(w, f32),
                      ((0, 0), (0, 0), (p, p), (p, p), (p, p)))

    avg3 = np.full((3, 3, 3), 1.0 / 27.0, f32)
    avg5 = np.full((5, 5, 5), 1.0 / 125.0, f32)
    experts = np.stack([
        np.asarray(w5, f32),
        pad_k(w3, 1),
        pad_k(w1, 2),
        pad_k(np.asarray(wa3, f32) * avg3[None, None], 1),
        np.asarray(wa5, f32) * avg5[None, None],
    ])                                               # [E, CO, CI, 5,5,5]
    wmix = np.einsum('eoidhw,neo->noidhw', experts, g).astype(f32)

    wls = []
    for n in range(N):
        wm = wmix[n]                                 # [CO, CI, 5,5,5]
        wl = np.zeros((NCHUNK, 128, CO), f32)
        t1 = wm.transpose(2, 1, 3, 4, 0)             # [kd, ci, kh, kw, co]
        for c in range(25):
            kh, kw = divmod(c, 5)
            wl[c] = t1[0:4, :, kh, kw, :].reshape(128, CO)
        t2 = wm[:, :, 4, 0:4, :].transpose(2, 1, 3, 0)  # [kh(j), ci, kw, co]
        for kw in range(5):
            wl[25 + kw] = t2[:, :, kw, :].reshape(128, CO)
        t3 = wm[:, :, 4, 4, :].transpose(2, 1, 0)    # [kw, ci, co]
        for kw in range(5):
            wl[30 + kw][0:CI] = t3[kw]
        wls.append(np.ascontiguousarray(
            wl.transpose(1, 0, 2).reshape(128, NCHUNK * CO)))

    gb = np.stack([np.asarray(gamma, f32), np.asarray(beta, f32)], axis=1)

    in_maps = []
    for c in range(NCORES):
        n, dh = divmod(c, 2)
        xpad = np.zeros((CI, NPLANES, PD, PD), f32)
        lo = dh * TD                # padded-plane base for this core
        # padded plane p (absolute) holds x depth p-2
        for p in range(NPLANES):
            src = lo + p - 2
            if 0 <= src < D:
                xpad[:, p, 2:2 + D, 2:2 + D] = x[n, :, src]
        in_maps.append({
            "xp": xpad.reshape(CI, NPLANES * PLANE),
            "wl": wls[n],
            "gb": gb,
        })
    return in_maps


def kernel(x, t, w5, w3, w1, wa3, wa5, gate_w, gate_b, gamma, beta):
    from concourse.bass_utils import run_bass_kernel_spmd

    if "nc" not in _CACHE:
        _CACHE["nc"] = _build_nc()
    nc = _CACHE["nc"]

    in_maps = _host_prep(x, t, w5, w3, w1, wa3, wa5, gate_w, gate_b,
                         gamma, beta)
    res = run_bass_kernel_spmd(nc, in_maps, list(range(NCORES)))

    out = np.empty((N, CO, D, D, D), np.float32)
    for c in range(NCORES):
        n, dh = divmod(c, 2)
        out[n, :, dh * TD:(dh + 1) * TD] = \
            res.results[c]["out"].reshape(CO, TD, D, D)
    return out


# revision 12
# speedup vs baseline: 1.5541x; 1.5541x over previous
"""Trainium2 Bass kernel for nn_Net_12902081757308 (moe_routing).

Mixture-of-expert-kernels 3D conv + InstanceNorm + Mish.

Math: gate g = softmax_E(t @ gate_w.T + gate_b) [N,E,CO]; per-sample mixed
5^3 kernel w[n] = sum_e g[n,e,co] * expert_e[co,ci,kd,kh,kw]; y = conv3d(x, w)
SAME; InstanceNorm3d (biased var, eps=1e-5, affine gamma/beta); Mish.

Sharding (8 cores): core c -> (sample n=c//2, depth-half dh=c%2). Each core
computes all CO=64 channels for 24 of 48 output depth planes. InstanceNorm
stats are reduced across the core pairs with a tiny AllReduce.

Device algorithm per core:
  Pass B (conv): contraction (CI*5^3=4000) split into 35 matmul chunks:
    25 chunks (kh,kw): K=128 rows (kd 0..3 x ci) against a depth-replicated
      SBUF buffer A (partition block j = padded plane d+j),
    5 chunks (kw): K=128 rows (kh 0..3 x ci) at kd=4 against an h-shifted
      buffer C (partition block j = plane d+4 shifted j rows),
    5 leftover chunks (kw): K=32 rows (ci) at kd=4,kh=4 from C block 0.
  All 35 accumulate into PSUM [64co, 8h, 48w] in f32 (inputs f32r).
  ACT Copy/Square with accum_out collect per-channel sum/sumsq.
  Pass C: AllReduce partial stats over core pairs; build affine s,b with
    s=gamma*rstd, b=beta-mu*s.
  Pass D: mish(z)=z*(q-1)/(q+1), q=(1+e^z)^2, z=y*s+b via ACT Exp/Square
    + DVE rational, plane-pairs packed to 128 partitions.
"""
import ml_dtypes
import numpy as np
from contextlib import ExitStack

E, CI, CO, K, T = 5, 32, 64, 5, 3
N, D = 4, 48
PD = D + 4            # padded spatial extent (52)
PLANE = PD * PD       # 2704
TD = D // 2           # output depths per core (24)
NPLANES = TD + 5      # input planes staged per core (28 + 1 guard)
VOL = D * D * D       # 110592 elements per (n, co) instance
OUTP = D * D          # 2304 per output plane
EPS = 1e-5
NCORES = 8
NCHUNK = 35
HTILES = (0, 16, 32)       # col-pair tiles: (h0, h0+8) per PSUM tile
NTILES = TD * len(HTILES)  # 72 pair-tiles

_CACHE = {}


def _build_nc():
    from concourse import bacc, mybir, tile

    dt = mybir.dt
    AFT = mybir.ActivationFunctionType

    nc = bacc.Bacc("TRN2", target_bir_lowering=False, debug=False,
                   num_devices=NCORES)
    xp_ap = nc.dram_tensor("xp", [CI, NPLANES * PLANE], dt.bfloat16,
                           kind="ExternalInput").ap()
    wl_ap = nc.dram_tensor("wl", [128, NCHUNK * CO], dt.bfloat16,
                           kind="ExternalInput").ap()
    gb_ap = nc.dram_tensor("gb", [CO, 2], dt.float32,
                           kind="ExternalInput").ap()
    out_ap = nc.dram_tensor("out", [CO, TD * OUTP], dt.float32,
                            kind="ExternalOutput").ap()

    with tile.TileContext(nc) as tc, ExitStack() as ctx:
        cpool = ctx.enter_context(tc.tile_pool(name="const", bufs=1))
        apool = ctx.enter_context(tc.tile_pool(name="abuf", bufs=2))
        cbpool = ctx.enter_context(tc.tile_pool(name="cbuf", bufs=2))
        ppool = ctx.enter_context(tc.tile_pool(name="psum", bufs=6,
                                               space="PSUM"))
        ypool = ctx.enter_context(tc.tile_pool(name="ysb", bufs=4))
        spool = ctx.enter_context(tc.tile_pool(name="stats", bufs=1))
        dpool = ctx.enter_context(tc.tile_pool(name="mish", bufs=2))
        drampool = ctx.enter_context(tc.tile_pool(name="dram", bufs=1,
                                                  space="DRAM"))

        wsb = cpool.tile([128, NCHUNK * CO], dt.bfloat16)
        nc.sync.dma_start(wsb[:], wl_ap[:])
        gbt = cpool.tile([CO, 2], dt.float32)
        nc.sync.dma_start(gbt[:], gb_ap[:])

        sums = spool.tile([128, NTILES], dt.float32)
        sumsq = spool.tile([128, NTILES], dt.float32)
        ydram = drampool.tile([CO, TD * OUTP], dt.float32)

        # ---- Pass B: conv + partial stats ----
        # Matmul free dim = contiguous 416-elem span (8 padded 52-rows) of
        # the flat plane; pad columns (w>=48) become garbage PSUM columns
        # that the extraction APs below skip. Two h-tiles (h0, h0+8) run
        # concurrently in the two PE column groups (tile_position 0/64),
        # doubling throughput for M=64.
        SPAN = 8 * PD  # 416
        ti = 0
        for d in range(TD):
            A = apool.tile([128, PLANE + 16], dt.bfloat16)
            for j in range(4):
                nc.sync.dma_start(
                    A[32 * j:32 * j + 32, 0:PLANE],
                    xp_ap[:, (d + j) * PLANE:(d + j + 1) * PLANE])
            Ct = cbpool.tile([128, PLANE + 16], dt.bfloat16)
            for j in range(4):
                off = (d + 4) * PLANE + j * PD
                nc.sync.dma_start(Ct[32 * j:32 * j + 32, 0:PLANE],
                                  xp_ap[:, off:off + PLANE])
            for h0 in HTILES:
                ps = ppool.tile([128, 8, PD], dt.float32)
                psf = ps[:].rearrange("p a b -> p (a b)")

                def mm(c, buf, off, k, start, stop):
                    for g, pb in ((0, 0), (1, 64)):
                        nc.tensor.matmul(
                            psf[pb:pb + CO, :],
                            wsb[0:k, c * CO:(c + 1) * CO],
                            buf[0:k, off + g * SPAN:off + (g + 1) * SPAN],
                            start=start, stop=stop,
                            tile_position=(0, pb))

                p0 = h0 * PD
                c = 0
                for kh in range(K):
                    for kw in range(K):
                        mm(c, A, p0 + kh * PD + kw, 128,
                           c == 0, False)
                        c += 1
                for kw in range(K):
                    mm(c, Ct, p0 + kw, 128, False, False)
                    c += 1
                for kw in range(K):
                    mm(c, Ct, p0 + 4 * PD + kw, 32,
                       False, c == NCHUNK - 1)
                    c += 1
                ysb = ypool.tile([128, 8, 48], dt.float32)
                nc.scalar.activation(ysb[:], ps[:, :, 0:48], AFT.Copy,
                                     accum_out=sums[:, ti:ti + 1])
                sqsb = ypool.tile([128, 8, 48], dt.float32)
                nc.scalar.activation(sqsb[:], ps[:, :, 0:48], AFT.Square,
                                     accum_out=sumsq[:, ti:ti + 1])
                base = d * OUTP + h0 * 48
                nc.sync.dma_start(
                    ydram[:, base:base + 384],
                    ysb[0:CO].rearrange("p a b -> p (a b)"))
                nc.sync.dma_start(
                    ydram[:, base + 384:base + 768],
                    ysb[CO:128].rearrange("p a b -> p (a b)"))
                ti += 1

        # ---- Pass C: stats reduce + AllReduce + affine ----
        red = spool.tile([128, 2], dt.float32)
        nc.vector.reduce_sum(red[:, 0:1], sums[:], axis=mybir.AxisListType.X)
        nc.vector.reduce_sum(red[:, 1:2], sumsq[:], axis=mybir.AxisListType.X)
        redhi = spool.tile([CO, 2], dt.float32)
        nc.sync.dma_start(redhi[:], red[CO:128, :])
        ccin = spool.tile([CO, 2], dt.float32)
        nc.vector.tensor_add(ccin[:], red[0:CO, :], redhi[:])
        cin_d = drampool.tile([CO, 2], dt.float32)
        cout_d = drampool.tile([CO, 2], dt.float32)
        nc.sync.dma_start(cin_d[:], ccin[:])
        nc.gpsimd.collective_compute(
            "AllReduce", mybir.AluOpType.add,
            replica_groups=[[0, 1], [2, 3], [4, 5], [6, 7]],
            ins=[cin_d.opt()], outs=[cout_d.opt()])
        st = spool.tile([CO, 2], dt.float32)
        nc.sync.dma_start(st[:], cout_d[:])

        mu = spool.tile([CO, 1], dt.float32)
        nc.vector.tensor_scalar_mul(mu[:], st[:, 0:1], 1.0 / VOL)
        m2 = spool.tile([CO, 1], dt.float32)
        nc.vector.tensor_scalar_mul(m2[:], st[:, 1:2], 1.0 / VOL)
        musq = spool.tile([CO, 1], dt.float32)
        nc.vector.tensor_mul(musq[:], mu[:], mu[:])
        var = spool.tile([CO, 1], dt.float32)
        nc.vector.tensor_scalar(var[:], m2[:], musq[:, 0:1], EPS,
                                mybir.AluOpType.subtract, mybir.AluOpType.add)
        std = spool.tile([CO, 1], dt.float32)
        nc.scalar.activation(std[:], var[:], AFT.Sqrt)
        rstd = spool.tile([CO, 1], dt.float32)
        nc.vector.reciprocal(rstd[:], std[:])
        sv = spool.tile([CO, 1], dt.float32)
        nc.vector.tensor_mul(sv[:], rstd[:], gbt[:, 0:1])
        mus = spool.tile([CO, 1], dt.float32)
        nc.vector.tensor_mul(mus[:], mu[:], sv[:])
        bv = spool.tile([CO, 1], dt.float32)
        nc.vector.tensor_sub(bv[:], gbt[:, 1:2], mus[:])
        # duplicate across both partition halves for 128-wide pass D
        sb2 = spool.tile([128, 2], dt.float32)
        nc.sync.dma_start(sb2[0:CO, 0:1], sv[:])
        nc.sync.dma_start(sb2[CO:128, 0:1], sv[:])
        nc.sync.dma_start(sb2[0:CO, 1:2], bv[:])
        nc.sync.dma_start(sb2[CO:128, 1:2], bv[:])

        # ---- Pass D: z=y*s+b; mish(z) = z*(q-1)/(q+1), q=(1+e^z)^2 ----
        for dd in range(TD // 2):
            yt = dpool.tile([128, OUTP], dt.float32)
            nc.sync.dma_start(yt[0:CO, :],
                              ydram[:, (2 * dd) * OUTP:(2 * dd + 1) * OUTP])
            nc.sync.dma_start(yt[CO:128, :],
                              ydram[:, (2 * dd + 1) * OUTP:(2 * dd + 2) * OUTP])
            e = dpool.tile([128, OUTP], dt.float32)
            nc.scalar.activation(e[:], yt[:], AFT.Exp,
                                 scale=sb2[:, 0:1], bias=sb2[:, 1:2])
            nc.scalar.activation(e[:], e[:], AFT.Square, bias=1.0)
            z = dpool.tile([128, OUTP], dt.float32)
            nc.vector.tensor_scalar(z[:], yt[:], sb2[:, 0:1], sb2[:, 1:2],
                                    mybir.AluOpType.mult, mybir.AluOpType.add)
            r = dpool.tile([128, OUTP], dt.float32)
            nc.vector.tensor_scalar_add(r[:], e[:], 1.0)
            nc.vector.reciprocal_approx_fast(r[:], r[:])
            nc.vector.scalar_tensor_tensor(z[:], e[:], -1.0, z[:],
                                           mybir.AluOpType.add,
                                           mybir.AluOpType.mult)
            nc.vector.tensor_mul(z[:], z[:], r[:])
            nc.sync.dma_start(out_ap[:, (2 * dd) * OUTP:(2 * dd + 1) * OUTP],
                              z[0:CO, :])
            nc.sync.dma_start(out_ap[:, (2 * dd + 1) * OUTP:(2 * dd + 2) * OUTP],
                              z[CO:128, :])
    nc.compile()
    return nc


def _host_prep(x, t, w5, w3, w1, wa3, wa5, gate_w, gate_b, gamma, beta):
    f32 = np.float32
    x = np.asarray(x, f32)
    t = np.asarray(t, f32)
    logits = t @ np.asarray(gate_w, f32).T + np.asarray(gate_b, f32)
    lg = logits.reshape(N, E, CO)
    lg = lg - lg.max(axis=1, keepdims=True)
    eg = np.exp(lg)
    g = eg / eg.sum(axis=1, keepdims=True)          # [N, E, CO]

    def pad_k(w, p):
        return np.pad(np.asarray(w, f32),
                      ((0, 0), (0, 0), (p, p), (p, p), (p, p)))

    avg3 = np.full((3, 3, 3), 1.0 / 27.0, f32)
    avg5 = np.full((5, 5, 5), 1.0 / 125.0, f32)
    experts = np.stack([
        np.asarray(w5, f32),
        pad_k(w3, 1),
        pad_k(w1, 2),
        pad_k(np.asarray(wa3, f32) * avg3[None, None], 1),
        np.asarray(wa5, f32) * avg5[None, None],
    ])                                               # [E, CO, CI, 5,5,5]
    wmix = np.einsum('eoidhw,neo->noidhw', experts, g).astype(f32)

    wls = []
    for n in range(N):
        wm = wmix[n]                                 # [CO, CI, 5,5,5]
        wl = np.zeros((NCHUNK, 128, CO), f32)
        t1 = wm.transpose(2, 1, 3, 4, 0)             # [kd, ci, kh, kw, co]
        for c in range(25):
            kh, kw = divmod(c, 5)
            wl[c] = t1[0:4, :, kh, kw, :].reshape(128, CO)
        t2 = wm[:, :, 4, 0:4, :].transpose(2, 1, 3, 0)  # [kh(j), ci, kw, co]
        for kw in range(5):
            wl[25 + kw] = t2[:, :, kw, :].reshape(128, CO)
        t3 = wm[:, :, 4, 4, :].transpose(2, 1, 0)    # [kw, ci, co]
        for kw in range(5):
            wl[30 + kw][0:CI] = t3[kw]
        wls.append(np.ascontiguousarray(
            wl.transpose(1, 0, 2).reshape(128, NCHUNK * CO))
            .astype(ml_dtypes.bfloat16))

    gb = np.stack([np.asarray(gamma, f32), np.asarray(beta, f32)], axis=1)

    in_maps = []
    for c in range(NCORES):
        n, dh = divmod(c, 2)
        xpad = np.zeros((CI, NPLANES, PD, PD), f32)
        lo = dh * TD                # padded-plane base for this core
        # padded plane p (absolute) holds x depth p-2
        for p in range(NPLANES):
            src = lo + p - 2
            if 0 <= src < D:
                xpad[:, p, 2:2 + D, 2:2 + D] = x[n, :, src]
        in_maps.append({
            "xp": xpad.reshape(CI, NPLANES * PLANE).astype(ml_dtypes.bfloat16),
            "wl": wls[n],
            "gb": gb,
        })
    return in_maps


def kernel(x, t, w5, w3, w1, wa3, wa5, gate_w, gate_b, gamma, beta):
    from concourse.bass_utils import run_bass_kernel_spmd

    if "nc" not in _CACHE:
        _CACHE["nc"] = _build_nc()
    nc = _CACHE["nc"]

    in_maps = _host_prep(x, t, w5, w3, w1, wa3, wa5, gate_w, gate_b,
                         gamma, beta)
    res = run_bass_kernel_spmd(nc, in_maps, list(range(NCORES)))

    out = np.empty((N, CO, D, D, D), np.float32)
    for c in range(NCORES):
        n, dh = divmod(c, 2)
        out[n, :, dh * TD:(dh + 1) * TD] = \
            res.results[c]["out"].reshape(CO, TD, D, D)
    return out


# revision 18
# speedup vs baseline: 1.8886x; 1.2152x over previous
"""Trainium2 Bass kernel for nn_Net_12902081757308 (moe_routing).

Mixture-of-expert-kernels 3D conv + InstanceNorm + Mish.

Math: gate g = softmax_E(t @ gate_w.T + gate_b) [N,E,CO]; per-sample mixed
5^3 kernel w[n] = sum_e g[n,e,co] * expert_e[co,ci,kd,kh,kw]; y = conv3d(x, w)
SAME; InstanceNorm3d (biased var, eps=1e-5, affine gamma/beta); Mish.

Sharding (8 cores): core c -> (sample n=c//2, depth-half dh=c%2). Each core
computes all CO=64 channels for 24 of 48 output depth planes. InstanceNorm
stats are reduced across the core pairs with a tiny AllReduce.

Device algorithm per core:
  Pass B (conv): contraction (CI*5^3=4000) split into 35 matmul chunks:
    25 chunks (kh,kw): K=128 rows (kd 0..3 x ci) against a depth-replicated
      SBUF buffer A (partition block j = padded plane d+j),
    5 chunks (kw): K=128 rows (kh 0..3 x ci) at kd=4 against an h-shifted
      buffer C (partition block j = plane d+4 shifted j rows),
    5 leftover chunks (kw): K=32 rows (ci) at kd=4,kh=4 from C block 0.
  All 35 accumulate into PSUM [64co, 8h, 48w] in f32 (inputs f32r).
  ACT Copy/Square with accum_out collect per-channel sum/sumsq.
  Pass C: AllReduce partial stats over core pairs; build affine s,b with
    s=gamma*rstd, b=beta-mu*s.
  Pass D: mish(z)=z*(q-1)/(q+1), q=(1+e^z)^2, z=y*s+b via ACT Exp/Square
    + DVE rational, plane-pairs packed to 128 partitions.
"""
import ml_dtypes
import numpy as np
from contextlib import ExitStack

E, CI, CO, K, T = 5, 32, 64, 5, 3
N, D = 4, 48
PD = D + 4            # padded spatial extent (52)
PLANE = PD * PD       # 2704
TD = D // 2           # output depths per core (24)
NPLANES = TD + 5      # input planes staged per core (28 + 1 guard)
VOL = D * D * D       # 110592 elements per (n, co) instance
OUTP = D * D          # 2304 per output plane
EPS = 1e-5
NCORES = 8
NCHUNK = 32
HTILES = (0, 16, 32)       # col-pair tiles: (h0, h0+8) per PSUM tile
NTILES = TD * len(HTILES)  # 72 pair-tiles

_CACHE = {}


def _build_nc():
    from concourse import bacc, mybir, tile

    dt = mybir.dt
    AFT = mybir.ActivationFunctionType

    nc = bacc.Bacc("TRN2", target_bir_lowering=False, debug=False,
                   num_devices=NCORES)
    xp_ap = nc.dram_tensor("xp", [CI, NPLANES * PLANE], dt.bfloat16,
                           kind="ExternalInput").ap()
    wl_ap = nc.dram_tensor("wl", [128, NCHUNK * CO], dt.bfloat16,
                           kind="ExternalInput").ap()
    gb_ap = nc.dram_tensor("gb", [CO, 2], dt.float32,
                           kind="ExternalInput").ap()
    out_ap = nc.dram_tensor("out", [CO, TD * OUTP], dt.float32,
                            kind="ExternalOutput").ap()

    with tile.TileContext(nc) as tc, ExitStack() as ctx:
        cpool = ctx.enter_context(tc.tile_pool(name="const", bufs=1))
        apool = ctx.enter_context(tc.tile_pool(name="abuf", bufs=3))
        cbpool = ctx.enter_context(tc.tile_pool(name="cbuf", bufs=3))
        wpool = ctx.enter_context(tc.tile_pool(name="wbuf", bufs=3))
        ppool = ctx.enter_context(tc.tile_pool(name="psum", bufs=8,
                                               space="PSUM"))
        ypool = ctx.enter_context(tc.tile_pool(name="ysb", bufs=4))
        spool = ctx.enter_context(tc.tile_pool(name="stats", bufs=1))
        dpool = ctx.enter_context(tc.tile_pool(name="mish", bufs=3))
        drampool = ctx.enter_context(tc.tile_pool(name="dram", bufs=1,
                                                  space="DRAM"))

        wsb = cpool.tile([128, NCHUNK * CO], dt.bfloat16)
        nc.sync.dma_start(wsb[:], wl_ap[:])
        gbt = cpool.tile([CO, 2], dt.float32)
        nc.sync.dma_start(gbt[:], gb_ap[:])

        sums = spool.tile([128, NTILES], dt.float32)
        sumsq = spool.tile([128, NTILES], dt.float32)
        ydram = drampool.tile([CO, TD * OUTP], dt.float32)

        # ---- Pass B: conv + partial stats ----
        # Matmul free dim = contiguous 416-elem span (8 padded 52-rows) of
        # the flat plane; pad columns (w>=48) become garbage PSUM columns
        # that the extraction APs below skip. Two h-tiles (h0, h0+8) run
        # concurrently in the two PE column groups (tile_position 0/64),
        # doubling throughput for M=64.
        SPAN = 8 * PD  # 416
        ti = 0
        for d in range(TD):
            A = apool.tile([128, PLANE + 16], dt.bfloat16)
            for j in range(4):
                nc.sync.dma_start(
                    A[32 * j:32 * j + 32, 0:PLANE],
                    xp_ap[:, (d + j) * PLANE:(d + j + 1) * PLANE])
            Ct = cbpool.tile([128, PLANE + 16], dt.bfloat16)
            for j in range(4):
                off = (d + 4) * PLANE + j * PD
                nc.sync.dma_start(Ct[32 * j:32 * j + 32, 0:PLANE],
                                  xp_ap[:, off:off + PLANE])
            # W4: plane d+4 shifted by (4 rows + j cols); block j serves
            # the kd=4,kh=4,kw=j leftovers as one K=128 chunk.
            W4 = wpool.tile([128, PLANE + 16], dt.bfloat16)
            for j in range(4):
                off = (d + 4) * PLANE + 4 * PD + j
                nc.sync.dma_start(W4[32 * j:32 * j + 32, 0:PLANE],
                                  xp_ap[:, off:off + PLANE])
            for h0 in HTILES:
                ps = ppool.tile([128, 8, PD], dt.float32)
                psf = ps[:].rearrange("p a b -> p (a b)")

                def mm(c, buf, off, k, start, stop):
                    for g, pb in ((0, 0), (1, 64)):
                        nc.tensor.matmul(
                            psf[pb:pb + CO, :],
                            wsb[0:k, c * CO:(c + 1) * CO],
                            buf[0:k, off + g * SPAN:off + (g + 1) * SPAN],
                            start=start, stop=stop,
                            tile_position=(0, pb))

                p0 = h0 * PD
                c = 0
                for kh in range(K):
                    for kw in range(K):
                        mm(c, A, p0 + kh * PD + kw, 128,
                           c == 0, False)
                        c += 1
                for kw in range(K):
                    mm(c, Ct, p0 + kw, 128, False, False)
                    c += 1
                mm(c, W4, p0, 128, False, False)        # kw 0..3
                c += 1
                mm(c, W4, p0 + 4, 32, False, True)      # kw=4
                c += 1
                ysb = ypool.tile([128, 8, 48], dt.float32)
                nc.scalar.activation(ysb[:], ps[:, :, 0:48], AFT.Copy,
                                     accum_out=sums[:, ti:ti + 1])
                sqsb = ypool.tile([128, 8, 48], dt.float32)
                nc.scalar.activation(sqsb[:], ps[:, :, 0:48], AFT.Square,
                                     accum_out=sumsq[:, ti:ti + 1])
                base = d * OUTP + h0 * 48
                nc.sync.dma_start(
                    ydram[:, base:base + 384],
                    ysb[0:CO].rearrange("p a b -> p (a b)"))
                nc.sync.dma_start(
                    ydram[:, base + 384:base + 768],
                    ysb[CO:128].rearrange("p a b -> p (a b)"))
                ti += 1

        # ---- Pass C: stats reduce + AllReduce + affine ----
        red = spool.tile([128, 2], dt.float32)
        nc.vector.reduce_sum(red[:, 0:1], sums[:], axis=mybir.AxisListType.X)
        nc.vector.reduce_sum(red[:, 1:2], sumsq[:], axis=mybir.AxisListType.X)
        redhi = spool.tile([CO, 2], dt.float32)
        nc.sync.dma_start(redhi[:], red[CO:128, :])
        ccin = spool.tile([CO, 2], dt.float32)
        nc.vector.tensor_add(ccin[:], red[0:CO, :], redhi[:])
        cin_d = drampool.tile([CO, 2], dt.float32)
        cout_d = drampool.tile([CO, 2], dt.float32)
        nc.sync.dma_start(cin_d[:], ccin[:])
        nc.gpsimd.collective_compute(
            "AllReduce", mybir.AluOpType.add,
            replica_groups=[[0, 1], [2, 3], [4, 5], [6, 7]],
            ins=[cin_d.opt()], outs=[cout_d.opt()])
        st = spool.tile([CO, 2], dt.float32)
        nc.sync.dma_start(st[:], cout_d[:])

        mu = spool.tile([CO, 1], dt.float32)
        nc.vector.tensor_scalar_mul(mu[:], st[:, 0:1], 1.0 / VOL)
        m2 = spool.tile([CO, 1], dt.float32)
        nc.vector.tensor_scalar_mul(m2[:], st[:, 1:2], 1.0 / VOL)
        musq = spool.tile([CO, 1], dt.float32)
        nc.vector.tensor_mul(musq[:], mu[:], mu[:])
        var = spool.tile([CO, 1], dt.float32)
        nc.vector.tensor_scalar(var[:], m2[:], musq[:, 0:1], EPS,
                                mybir.AluOpType.subtract, mybir.AluOpType.add)
        std = spool.tile([CO, 1], dt.float32)
        nc.scalar.activation(std[:], var[:], AFT.Sqrt)
        rstd = spool.tile([CO, 1], dt.float32)
        nc.vector.reciprocal(rstd[:], std[:])
        sv = spool.tile([CO, 1], dt.float32)
        nc.vector.tensor_mul(sv[:], rstd[:], gbt[:, 0:1])
        mus = spool.tile([CO, 1], dt.float32)
        nc.vector.tensor_mul(mus[:], mu[:], sv[:])
        bv = spool.tile([CO, 1], dt.float32)
        nc.vector.tensor_sub(bv[:], gbt[:, 1:2], mus[:])
        # duplicate across both partition halves for 128-wide pass D
        sb2 = spool.tile([128, 2], dt.float32)
        nc.sync.dma_start(sb2[0:CO, 0:1], sv[:])
        nc.sync.dma_start(sb2[CO:128, 0:1], sv[:])
        nc.sync.dma_start(sb2[0:CO, 1:2], bv[:])
        nc.sync.dma_start(sb2[CO:128, 1:2], bv[:])

        # ---- Pass D: z=y*s+b; mish(z) = z*(q-1)/(q+1), q=(1+e^z)^2 ----
        for dd in range(TD // 2):
            yt = dpool.tile([128, OUTP], dt.float32)
            nc.sync.dma_start(yt[0:CO, :],
                              ydram[:, (2 * dd) * OUTP:(2 * dd + 1) * OUTP])
            nc.sync.dma_start(yt[CO:128, :],
                              ydram[:, (2 * dd + 1) * OUTP:(2 * dd + 2) * OUTP])
            e = dpool.tile([128, OUTP], dt.float32)
            nc.scalar.activation(e[:], yt[:], AFT.Exp,
                                 scale=sb2[:, 0:1], bias=sb2[:, 1:2])
            nc.scalar.activation(e[:], e[:], AFT.Square, bias=1.0)
            z = dpool.tile([128, OUTP], dt.float32)
            nc.scalar.activation(z[:], yt[:], AFT.Identity,
                                 scale=sb2[:, 0:1], bias=sb2[:, 1:2])
            r = dpool.tile([128, OUTP], dt.float32)
            nc.vector.tensor_scalar_add(r[:], e[:], 1.0)
            nc.vector.reciprocal_approx_fast(r[:], r[:])
            nc.vector.scalar_tensor_tensor(z[:], e[:], -1.0, z[:],
                                           mybir.AluOpType.add,
                                           mybir.AluOpType.mult)
            nc.vector.tensor_mul(z[:], z[:], r[:])
            nc.sync.dma_start(out_ap[:, (2 * dd) * OUTP:(2 * dd + 1) * OUTP],
                              z[0:CO, :])
            nc.sync.dma_start(out_ap[:, (2 * dd + 1) * OUTP:(2 * dd + 2) * OUTP],
                              z[CO:128, :])
    nc.compile()
    return nc


def _host_prep(x, t, w5, w3, w1, wa3, wa5, gate_w, gate_b, gamma, beta):
    f32 = np.float32
    x = np.asarray(x, f32)
    t = np.asarray(t, f32)
    logits = t @ np.asarray(gate_w, f32).T + np.asarray(gate_b, f32)
    lg = logits.reshape(N, E, CO)
    lg = lg - lg.max(axis=1, keepdims=True)
    eg = np.exp(lg)
    g = eg / eg.sum(axis=1, keepdims=True)          # [N, E, CO]

    def pad_k(w, p):
        return np.pad(np.asarray(w, f32),
                      ((0, 0), (0, 0), (p, p), (p, p), (p, p)))

    avg3 = np.full((3, 3, 3), 1.0 / 27.0, f32)
    avg5 = np.full((5, 5, 5), 1.0 / 125.0, f32)
    experts = np.stack([
        np.asarray(w5, f32),
        pad_k(w3, 1),
        pad_k(w1, 2),
        pad_k(np.asarray(wa3, f32) * avg3[None, None], 1),
        np.asarray(wa5, f32) * avg5[None, None],
    ])                                               # [E, CO, CI, 5,5,5]
    wmix = np.einsum('eoidhw,neo->noidhw', experts, g).astype(f32)

    wls = []
    for n in range(N):
        wm = wmix[n]                                 # [CO, CI, 5,5,5]
        wl = np.zeros((NCHUNK, 128, CO), f32)
        t1 = wm.transpose(2, 1, 3, 4, 0)             # [kd, ci, kh, kw, co]
        for c in range(25):
            kh, kw = divmod(c, 5)
            wl[c] = t1[0:4, :, kh, kw, :].reshape(128, CO)
        t2 = wm[:, :, 4, 0:4, :].transpose(2, 1, 3, 0)  # [kh(j), ci, kw, co]
        for kw in range(5):
            wl[25 + kw] = t2[:, :, kw, :].reshape(128, CO)
        t3 = wm[:, :, 4, 4, :].transpose(2, 1, 0)    # [kw, ci, co]
        wl[30] = t3[0:4].reshape(128, CO)            # kw 0..3 on row blocks
        wl[31][0:CI] = t3[4]                         # kw=4, K=32
        wls.append(np.ascontiguousarray(
            wl.transpose(1, 0, 2).reshape(128, NCHUNK * CO))
            .astype(ml_dtypes.bfloat16))

    gb = np.stack([np.asarray(gamma, f32), np.asarray(beta, f32)], axis=1)

    in_maps = []
    for c in range(NCORES):
        n, dh = divmod(c, 2)
        xpad = np.zeros((CI, NPLANES, PD, PD), f32)
        lo = dh * TD                # padded-plane base for this core
        # padded plane p (absolute) holds x depth p-2
        for p in range(NPLANES):
            src = lo + p - 2
            if 0 <= src < D:
                xpad[:, p, 2:2 + D, 2:2 + D] = x[n, :, src]
        in_maps.append({
            "xp": xpad.reshape(CI, NPLANES * PLANE).astype(ml_dtypes.bfloat16),
            "wl": wls[n],
            "gb": gb,
        })
    return in_maps


def kernel(x, t, w5, w3, w1, wa3, wa5, gate_w, gate_b, gamma, beta):
    from concourse.bass_utils import run_bass_kernel_spmd

    if "nc" not in _CACHE:
        _CACHE["nc"] = _build_nc()
    nc = _CACHE["nc"]

    in_maps = _host_prep(x, t, w5, w3, w1, wa3, wa5, gate_w, gate_b,
                         gamma, beta)
    res = run_bass_kernel_spmd(nc, in_maps, list(range(NCORES)))

    out = np.empty((N, CO, D, D, D), np.float32)
    for c in range(NCORES):
        n, dh = divmod(c, 2)
        out[n, :, dh * TD:(dh + 1) * TD] = \
            res.results[c]["out"].reshape(CO, TD, D, D)
    return out


# revision 22
# speedup vs baseline: 1.8912x; 1.0014x over previous
"""Trainium2 Bass kernel for nn_Net_12902081757308 (moe_routing).

Mixture-of-expert-kernels 3D conv + InstanceNorm + Mish.

Math: gate g = softmax_E(t @ gate_w.T + gate_b) [N,E,CO]; per-sample mixed
5^3 kernel w[n] = sum_e g[n,e,co] * expert_e[co,ci,kd,kh,kw]; y = conv3d(x, w)
SAME; InstanceNorm3d (biased var, eps=1e-5, affine gamma/beta); Mish.

Sharding (8 cores): core c -> (sample n=c//2, depth-half dh=c%2). Each core
computes all CO=64 channels for 24 of 48 output depth planes. InstanceNorm
stats are reduced across the core pairs with a tiny AllReduce.

Device algorithm per core:
  Pass B (conv): contraction (CI*5^3=4000) split into 35 matmul chunks:
    25 chunks (kh,kw): K=128 rows (kd 0..3 x ci) against a depth-replicated
      SBUF buffer A (partition block j = padded plane d+j),
    5 chunks (kw): K=128 rows (kh 0..3 x ci) at kd=4 against an h-shifted
      buffer C (partition block j = plane d+4 shifted j rows),
    5 leftover chunks (kw): K=32 rows (ci) at kd=4,kh=4 from C block 0.
  All 35 accumulate into PSUM [64co, 8h, 48w] in f32 (inputs f32r).
  ACT Copy/Square with accum_out collect per-channel sum/sumsq.
  Pass C: AllReduce partial stats over core pairs; build affine s,b with
    s=gamma*rstd, b=beta-mu*s.
  Pass D: mish(z)=z*(q-1)/(q+1), q=(1+e^z)^2, z=y*s+b via ACT Exp/Square
    + DVE rational, plane-pairs packed to 128 partitions.
"""
import ml_dtypes
import numpy as np
from contextlib import ExitStack

E, CI, CO, K, T = 5, 32, 64, 5, 3
N, D = 4, 48
PD = D + 4            # padded spatial extent (52)
PLANE = PD * PD       # 2704
TD = D // 2           # output depths per core (24)
NPLANES = TD + 5      # input planes staged per core (28 + 1 guard)
VOL = D * D * D       # 110592 elements per (n, co) instance
OUTP = D * D          # 2304 per output plane
EPS = 1e-5
NCORES = 8
NCHUNK = 32
HTILES = (0, 16, 32)       # col-pair tiles: (h0, h0+8) per PSUM tile
NTILES = TD * len(HTILES)  # 72 pair-tiles

_CACHE = {}


def _build_nc():
    from concourse import bacc, mybir, tile

    dt = mybir.dt
    AFT = mybir.ActivationFunctionType

    nc = bacc.Bacc("TRN2", target_bir_lowering=False, debug=False,
                   num_devices=NCORES)
    xp_ap = nc.dram_tensor("xp", [CI, NPLANES * PLANE], dt.bfloat16,
                           kind="ExternalInput").ap()
    wl_ap = nc.dram_tensor("wl", [128, NCHUNK * CO], dt.bfloat16,
                           kind="ExternalInput").ap()
    gb_ap = nc.dram_tensor("gb", [CO, 2], dt.float32,
                           kind="ExternalInput").ap()
    out_ap = nc.dram_tensor("out", [CO, TD * OUTP], dt.float32,
                            kind="ExternalOutput").ap()

    with tile.TileContext(nc) as tc, ExitStack() as ctx:
        cpool = ctx.enter_context(tc.tile_pool(name="const", bufs=1))
        apool = ctx.enter_context(tc.tile_pool(name="abuf", bufs=3))
        cbpool = ctx.enter_context(tc.tile_pool(name="cbuf", bufs=3))
        wpool = ctx.enter_context(tc.tile_pool(name="wbuf", bufs=3))
        ppool = ctx.enter_context(tc.tile_pool(name="psum", bufs=8,
                                               space="PSUM"))
        ypool = ctx.enter_context(tc.tile_pool(name="ysb", bufs=4))
        spool = ctx.enter_context(tc.tile_pool(name="stats", bufs=1))
        dpool = ctx.enter_context(tc.tile_pool(name="mish", bufs=3))
        drampool = ctx.enter_context(tc.tile_pool(name="dram", bufs=1,
                                                  space="DRAM"))

        wsb = cpool.tile([128, NCHUNK * CO], dt.bfloat16)
        nc.sync.dma_start(wsb[:], wl_ap[:])
        gbt = cpool.tile([CO, 2], dt.float32)
        nc.sync.dma_start(gbt[:], gb_ap[:])

        sums = spool.tile([128, NTILES], dt.float32)
        sumsq = spool.tile([128, NTILES], dt.float32)
        ydram = drampool.tile([CO, TD * OUTP], dt.float32)

        # ---- Pass B: conv + partial stats ----
        # Matmul free dim = contiguous 416-elem span (8 padded 52-rows) of
        # the flat plane; pad columns (w>=48) become garbage PSUM columns
        # that the extraction APs below skip. Two h-tiles (h0, h0+8) run
        # concurrently in the two PE column groups (tile_position 0/64),
        # doubling throughput for M=64.
        SPAN = 8 * PD  # 416
        ti = 0
        for d in range(TD):
            A = apool.tile([128, PLANE + 16], dt.bfloat16)
            for j in range(4):
                nc.sync.dma_start(
                    A[32 * j:32 * j + 32, 0:PLANE],
                    xp_ap[:, (d + j) * PLANE:(d + j + 1) * PLANE])
            Ct = cbpool.tile([128, PLANE + 16], dt.bfloat16)
            for j in range(4):
                off = (d + 4) * PLANE + j * PD
                nc.sync.dma_start(Ct[32 * j:32 * j + 32, 0:PLANE],
                                  xp_ap[:, off:off + PLANE])
            # W4: plane d+4 shifted by (4 rows + j cols); block j serves
            # the kd=4,kh=4,kw=j leftovers as one K=128 chunk.
            W4 = wpool.tile([128, PLANE + 16], dt.bfloat16)
            for j in range(4):
                off = (d + 4) * PLANE + 4 * PD + j
                nc.sync.dma_start(W4[32 * j:32 * j + 32, 0:PLANE],
                                  xp_ap[:, off:off + PLANE])
            for h0 in HTILES:
                ps = ppool.tile([128, 8, PD], dt.float32)
                psf = ps[:].rearrange("p a b -> p (a b)")

                def mm(c, buf, off, k, start, stop):
                    for g, pb in ((0, 0), (1, 64)):
                        nc.tensor.matmul(
                            psf[pb:pb + CO, :],
                            wsb[0:k, c * CO:(c + 1) * CO],
                            buf[0:k, off + g * SPAN:off + (g + 1) * SPAN],
                            start=start, stop=stop,
                            tile_position=(0, pb))

                p0 = h0 * PD
                c = 0
                for kh in range(K):
                    for kw in range(K):
                        mm(c, A, p0 + kh * PD + kw, 128,
                           c == 0, False)
                        c += 1
                for kw in range(K):
                    mm(c, Ct, p0 + kw, 128, False, False)
                    c += 1
                mm(c, W4, p0, 128, False, False)        # kw 0..3
                c += 1
                mm(c, W4, p0 + 4, 32, False, True)      # kw=4
                c += 1
                ysb = ypool.tile([128, 8, 48], dt.float32)
                nc.scalar.activation(ysb[:], ps[:, :, 0:48], AFT.Copy,
                                     accum_out=sums[:, ti:ti + 1])
                sqsb = ypool.tile([128, 8, 48], dt.float32)
                nc.scalar.activation(sqsb[:], ps[:, :, 0:48], AFT.Square,
                                     accum_out=sumsq[:, ti:ti + 1])
                base = d * OUTP + h0 * 48
                nc.sync.dma_start(
                    ydram[:, base:base + 384],
                    ysb[0:CO].rearrange("p a b -> p (a b)"))
                nc.sync.dma_start(
                    ydram[:, base + 384:base + 768],
                    ysb[CO:128].rearrange("p a b -> p (a b)"))
                ti += 1

        # ---- Pass C: stats reduce + AllReduce + affine ----
        red = spool.tile([128, 2], dt.float32)
        nc.vector.reduce_sum(red[:, 0:1], sums[:], axis=mybir.AxisListType.X)
        nc.vector.reduce_sum(red[:, 1:2], sumsq[:], axis=mybir.AxisListType.X)
        redhi = spool.tile([CO, 2], dt.float32)
        nc.sync.dma_start(redhi[:], red[CO:128, :])
        ccin = spool.tile([CO, 2], dt.float32)
        nc.vector.tensor_add(ccin[:], red[0:CO, :], redhi[:])
        cin_d = drampool.tile([CO, 2], dt.float32)
        cout_d = drampool.tile([CO, 2], dt.float32)
        nc.sync.dma_start(cin_d[:], ccin[:])
        nc.gpsimd.collective_compute(
            "AllReduce", mybir.AluOpType.add,
            replica_groups=[[0, 1], [2, 3], [4, 5], [6, 7]],
            ins=[cin_d.opt()], outs=[cout_d.opt()])
        st = spool.tile([CO, 2], dt.float32)
        nc.sync.dma_start(st[:], cout_d[:])

        stv = spool.tile([CO, 2], dt.float32)
        nc.vector.tensor_scalar_mul(stv[:], st[:], 1.0 / VOL)  # [mu, m2]
        mu = stv[:, 0:1]
        musq = spool.tile([CO, 1], dt.float32)
        nc.vector.tensor_mul(musq[:], mu, mu)
        var = spool.tile([CO, 1], dt.float32)
        nc.vector.tensor_scalar(var[:], stv[:, 1:2], musq[:, 0:1], EPS,
                                mybir.AluOpType.subtract, mybir.AluOpType.add)
        std = spool.tile([CO, 1], dt.float32)
        nc.scalar.activation(std[:], var[:], AFT.Sqrt)
        rstd = spool.tile([CO, 1], dt.float32)
        nc.vector.reciprocal(rstd[:], std[:])
        sb = spool.tile([CO, 2], dt.float32)
        nc.vector.tensor_mul(sb[:, 0:1], rstd[:], gbt[:, 0:1])
        mus = spool.tile([CO, 1], dt.float32)
        nc.vector.tensor_mul(mus[:], mu, sb[:, 0:1])
        nc.vector.tensor_sub(sb[:, 1:2], gbt[:, 1:2], mus[:])
        # duplicate across both partition halves for 128-wide pass D
        sb2 = spool.tile([128, 2], dt.float32)
        nc.sync.dma_start(sb2[0:CO, :], sb[:])
        nc.sync.dma_start(sb2[CO:128, :], sb[:])

        # ---- Pass D: z=y*s+b; mish(z) = z*(q-1)/(q+1), q=(1+e^z)^2 ----
        for dd in range(TD // 2):
            yt = dpool.tile([128, OUTP], dt.float32)
            nc.sync.dma_start(yt[0:CO, :],
                              ydram[:, (2 * dd) * OUTP:(2 * dd + 1) * OUTP])
            nc.sync.dma_start(yt[CO:128, :],
                              ydram[:, (2 * dd + 1) * OUTP:(2 * dd + 2) * OUTP])
            e = dpool.tile([128, OUTP], dt.float32)
            nc.scalar.activation(e[:], yt[:], AFT.Exp,
                                 scale=sb2[:, 0:1], bias=sb2[:, 1:2])
            nc.scalar.activation(e[:], e[:], AFT.Square, bias=1.0)
            z = dpool.tile([128, OUTP], dt.float32)
            nc.scalar.activation(z[:], yt[:], AFT.Identity,
                                 scale=sb2[:, 0:1], bias=sb2[:, 1:2])
            r = dpool.tile([128, OUTP], dt.float32)
            nc.vector.tensor_scalar_add(r[:], e[:], 1.0)
            nc.vector.reciprocal_approx_fast(r[:], r[:])
            nc.vector.scalar_tensor_tensor(z[:], e[:], -1.0, z[:],
                                           mybir.AluOpType.add,
                                           mybir.AluOpType.mult)
            nc.vector.tensor_mul(z[:], z[:], r[:])
            nc.sync.dma_start(out_ap[:, (2 * dd) * OUTP:(2 * dd + 1) * OUTP],
                              z[0:CO, :])
            nc.sync.dma_start(out_ap[:, (2 * dd + 1) * OUTP:(2 * dd + 2) * OUTP],
                              z[CO:128, :])
    nc.compile()
    return nc


def _host_prep(x, t, w5, w3, w1, wa3, wa5, gate_w, gate_b, gamma, beta):
    f32 = np.float32
    x = np.asarray(x, f32)
    t = np.asarray(t, f32)
    logits = t @ np.asarray(gate_w, f32).T + np.asarray(gate_b, f32)
    lg = logits.reshape(N, E, CO)
    lg = lg - lg.max(axis=1, keepdims=True)
    eg = np.exp(lg)
    g = eg / eg.sum(axis=1, keepdims=True)          # [N, E, CO]

    def pad_k(w, p):
        return np.pad(np.asarray(w, f32),
                      ((0, 0), (0, 0), (p, p), (p, p), (p, p)))

    avg3 = np.full((3, 3, 3), 1.0 / 27.0, f32)
    avg5 = np.full((5, 5, 5), 1.0 / 125.0, f32)
    experts = np.stack([
        np.asarray(w5, f32),
        pad_k(w3, 1),
        pad_k(w1, 2),
        pad_k(np.asarray(wa3, f32) * avg3[None, None], 1),
        np.asarray(wa5, f32) * avg5[None, None],
    ])                                               # [E, CO, CI, 5,5,5]
    wmix = np.einsum('eoidhw,neo->noidhw', experts, g).astype(f32)

    wls = []
    for n in range(N):
        wm = wmix[n]                                 # [CO, CI, 5,5,5]
        wl = np.zeros((NCHUNK, 128, CO), f32)
        t1 = wm.transpose(2, 1, 3, 4, 0)             # [kd, ci, kh, kw, co]
        for c in range(25):
            kh, kw = divmod(c, 5)
            wl[c] = t1[0:4, :, kh, kw, :].reshape(128, CO)
        t2 = wm[:, :, 4, 0:4, :].transpose(2, 1, 3, 0)  # [kh(j), ci, kw, co]
        for kw in range(5):
            wl[25 + kw] = t2[:, :, kw, :].reshape(128, CO)
        t3 = wm[:, :, 4, 4, :].transpose(2, 1, 0)    # [kw, ci, co]
        wl[30] = t3[0:4].reshape(128, CO)            # kw 0..3 on row blocks
        wl[31][0:CI] = t3[4]                         # kw=4, K=32
        wls.append(np.ascontiguousarray(
            wl.transpose(1, 0, 2).reshape(128, NCHUNK * CO))
            .astype(ml_dtypes.bfloat16))

    gb = np.stack([np.asarray(gamma, f32), np.asarray(beta, f32)], axis=1)

    in_maps = []
    for c in range(NCORES):
        n, dh = divmod(c, 2)
        xpad = np.zeros((CI, NPLANES, PD, PD), f32)
        lo = dh * TD                # padded-plane base for this core
        # padded plane p (absolute) holds x depth p-2
        for p in range(NPLANES):
            src = lo + p - 2
            if 0 <= src < D:
                xpad[:, p, 2:2 + D, 2:2 + D] = x[n, :, src]
        in_maps.append({
            "xp": xpad.reshape(CI, NPLANES * PLANE).astype(ml_dtypes.bfloat16),
            "wl": wls[n],
            "gb": gb,
        })
    return in_maps


def kernel(x, t, w5, w3, w1, wa3, wa5, gate_w, gate_b, gamma, beta):
    from concourse.bass_utils import run_bass_kernel_spmd

    if "nc" not in _CACHE:
        _CACHE["nc"] = _build_nc()
    nc = _CACHE["nc"]

    in_maps = _host_prep(x, t, w5, w3, w1, wa3, wa5, gate_w, gate_b,
                         gamma, beta)
    res = run_bass_kernel_spmd(nc, in_maps, list(range(NCORES)))

    out = np.empty((N, CO, D, D, D), np.float32)
    for c in range(NCORES):
        n, dh = divmod(c, 2)
        out[n, :, dh * TD:(dh + 1) * TD] = \
            res.results[c]["out"].reshape(CO, TD, D, D)
    return out


# revision 24
# speedup vs baseline: 1.9050x; 1.0073x over previous
"""Trainium2 Bass kernel for nn_Net_12902081757308 (moe_routing).

Mixture-of-expert-kernels 3D conv + InstanceNorm + Mish.

Math: gate g = softmax_E(t @ gate_w.T + gate_b) [N,E,CO]; per-sample mixed
5^3 kernel w[n] = sum_e g[n,e,co] * expert_e[co,ci,kd,kh,kw]; y = conv3d(x, w)
SAME; InstanceNorm3d (biased var, eps=1e-5, affine gamma/beta); Mish.

Sharding (8 cores): core c -> (sample n=c//2, depth-half dh=c%2). Each core
computes all CO=64 channels for 24 of 48 output depth planes. InstanceNorm
stats are reduced across the core pairs with a tiny AllReduce.

Device algorithm per core:
  Pass B (conv): contraction (CI*5^3=4000) split into 35 matmul chunks:
    25 chunks (kh,kw): K=128 rows (kd 0..3 x ci) against a depth-replicated
      SBUF buffer A (partition block j = padded plane d+j),
    5 chunks (kw): K=128 rows (kh 0..3 x ci) at kd=4 against an h-shifted
      buffer C (partition block j = plane d+4 shifted j rows),
    5 leftover chunks (kw): K=32 rows (ci) at kd=4,kh=4 from C block 0.
  All 35 accumulate into PSUM [64co, 8h, 48w] in f32 (inputs f32r).
  ACT Copy/Square with accum_out collect per-channel sum/sumsq.
  Pass C: AllReduce partial stats over core pairs; build affine s,b with
    s=gamma*rstd, b=beta-mu*s.
  Pass D: mish(z)=z*(q-1)/(q+1), q=(1+e^z)^2, z=y*s+b via ACT Exp/Square
    + DVE rational, plane-pairs packed to 128 partitions.
"""
import ml_dtypes
import numpy as np
from contextlib import ExitStack

E, CI, CO, K, T = 5, 32, 64, 5, 3
N, D = 4, 48
PD = D + 4            # padded spatial extent (52)
PLANE = PD * PD       # 2704
TD = D // 2           # output depths per core (24)
NPLANES = TD + 5      # input planes staged per core (28 + 1 guard)
VOL = D * D * D       # 110592 elements per (n, co) instance
OUTP = D * D          # 2304 per output plane
EPS = 1e-5
NCORES = 8
NCHUNK = 32
HTILES = (0, 16, 32)       # col-pair tiles: (h0, h0+8) per PSUM tile
NTILES = TD * len(HTILES)  # 72 pair-tiles

_CACHE = {}


def _build_nc():
    from concourse import bacc, mybir, tile

    dt = mybir.dt
    AFT = mybir.ActivationFunctionType

    nc = bacc.Bacc("TRN2", target_bir_lowering=False, debug=False,
                   num_devices=NCORES)
    xp_ap = nc.dram_tensor("xp", [CI, NPLANES * PLANE], dt.bfloat16,
                           kind="ExternalInput").ap()
    wl_ap = nc.dram_tensor("wl", [128, NCHUNK * CO], dt.bfloat16,
                           kind="ExternalInput").ap()
    gb_ap = nc.dram_tensor("gb", [CO, 2], dt.float32,
                           kind="ExternalInput").ap()
    out_ap = nc.dram_tensor("out", [CO, TD * OUTP], dt.float32,
                            kind="ExternalOutput").ap()

    with tile.TileContext(nc) as tc, ExitStack() as ctx:
        cpool = ctx.enter_context(tc.tile_pool(name="const", bufs=1))
        apool = ctx.enter_context(tc.tile_pool(name="abuf", bufs=3))
        cbpool = ctx.enter_context(tc.tile_pool(name="cbuf", bufs=3))
        wpool = ctx.enter_context(tc.tile_pool(name="wbuf", bufs=3))
        ppool = ctx.enter_context(tc.tile_pool(name="psum", bufs=8,
                                               space="PSUM"))
        ypool = ctx.enter_context(tc.tile_pool(name="ysb", bufs=4))
        spool = ctx.enter_context(tc.tile_pool(name="stats", bufs=1))
        dpool = ctx.enter_context(tc.tile_pool(name="mish", bufs=3))
        drampool = ctx.enter_context(tc.tile_pool(name="dram", bufs=1,
                                                  space="DRAM"))

        wsb = cpool.tile([128, NCHUNK * CO], dt.bfloat16)
        nc.sync.dma_start(wsb[:], wl_ap[:])
        gbt = cpool.tile([CO, 2], dt.float32)
        nc.sync.dma_start(gbt[:], gb_ap[:])

        sums = spool.tile([128, NTILES], dt.float32)
        sumsq = spool.tile([128, NTILES], dt.float32)
        ydram = drampool.tile([CO, TD * OUTP], dt.float32)

        # ---- Pass B: conv + partial stats ----
        # rhs = strided [8 rows x 48 @ stride 52] windows of the flat padded
        # plane (bf16 sustains 1 col/cycle on strided reads). Two h-tiles
        # (h0, h0+8) run concurrently in the two PE column groups
        # (tile_position 0/64), doubling throughput for M=64.
        ti = 0
        for d in range(TD):
            A = apool.tile([128, PD, PD], dt.bfloat16)
            for j in range(4):
                nc.sync.dma_start(
                    A[32 * j:32 * j + 32],
                    xp_ap[:, (d + j) * PLANE:(d + j + 1) * PLANE])
            Ct = cbpool.tile([128, PD, PD], dt.bfloat16)
            for j in range(4):
                off = (d + 4) * PLANE + j * PD
                nc.sync.dma_start(Ct[32 * j:32 * j + 32],
                                  xp_ap[:, off:off + PLANE])
            # W4: plane d+4 shifted by (4 rows + j cols); block j serves
            # the kd=4,kh=4,kw=j leftovers as one K=128 chunk.
            W4 = wpool.tile([128, PD, PD], dt.bfloat16)
            for j in range(4):
                off = (d + 4) * PLANE + 4 * PD + j
                nc.sync.dma_start(W4[32 * j:32 * j + 32],
                                  xp_ap[:, off:off + PLANE])
            for h0 in HTILES:
                ps = ppool.tile([128, 8, 48], dt.float32)

                def mm(c, buf, ro, co_, k, start, stop):
                    for g, pb in ((0, 0), (1, 64)):
                        r0 = ro + 8 * g
                        nc.tensor.matmul(
                            ps[pb:pb + CO],
                            wsb[0:k, c * CO:(c + 1) * CO],
                            buf[0:k, r0:r0 + 8, co_:co_ + 48],
                            start=start, stop=stop,
                            tile_position=(0, pb))

                c = 0
                for kh in range(K):
                    for kw in range(K):
                        mm(c, A, h0 + kh, kw, 128, c == 0, False)
                        c += 1
                for kw in range(K):
                    mm(c, Ct, h0, kw, 128, False, False)
                    c += 1
                mm(c, W4, h0, 0, 128, False, False)   # kw 0..3
                c += 1
                mm(c, W4, h0, 4, 32, False, True)     # kw=4
                c += 1
                ysb = ypool.tile([128, 8, 48], dt.float32)
                nc.scalar.activation(ysb[:], ps[:], AFT.Copy,
                                     accum_out=sums[:, ti:ti + 1])
                sqsb = ypool.tile([128, 8, 48], dt.float32)
                nc.scalar.activation(sqsb[:], ps[:], AFT.Square,
                                     accum_out=sumsq[:, ti:ti + 1])
                base = d * OUTP + h0 * 48
                nc.sync.dma_start(
                    ydram[:, base:base + 384],
                    ysb[0:CO].rearrange("p a b -> p (a b)"))
                nc.sync.dma_start(
                    ydram[:, base + 384:base + 768],
                    ysb[CO:128].rearrange("p a b -> p (a b)"))
                ti += 1

        # ---- Pass C: stats reduce + AllReduce + affine ----
        red = spool.tile([128, 2], dt.float32)
        nc.vector.reduce_sum(red[:, 0:1], sums[:], axis=mybir.AxisListType.X)
        nc.vector.reduce_sum(red[:, 1:2], sumsq[:], axis=mybir.AxisListType.X)
        redhi = spool.tile([CO, 2], dt.float32)
        nc.sync.dma_start(redhi[:], red[CO:128, :])
        ccin = spool.tile([CO, 2], dt.float32)
        nc.vector.tensor_add(ccin[:], red[0:CO, :], redhi[:])
        cin_d = drampool.tile([CO, 2], dt.float32)
        cout_d = drampool.tile([CO, 2], dt.float32)
        nc.sync.dma_start(cin_d[:], ccin[:])
        nc.gpsimd.collective_compute(
            "AllReduce", mybir.AluOpType.add,
            replica_groups=[[0, 1], [2, 3], [4, 5], [6, 7]],
            ins=[cin_d.opt()], outs=[cout_d.opt()])
        st = spool.tile([CO, 2], dt.float32)
        nc.sync.dma_start(st[:], cout_d[:])

        stv = spool.tile([CO, 2], dt.float32)
        nc.vector.tensor_scalar_mul(stv[:], st[:], 1.0 / VOL)  # [mu, m2]
        mu = stv[:, 0:1]
        musq = spool.tile([CO, 1], dt.float32)
        nc.vector.tensor_mul(musq[:], mu, mu)
        var = spool.tile([CO, 1], dt.float32)
        nc.vector.tensor_scalar(var[:], stv[:, 1:2], musq[:, 0:1], EPS,
                                mybir.AluOpType.subtract, mybir.AluOpType.add)
        std = spool.tile([CO, 1], dt.float32)
        nc.scalar.activation(std[:], var[:], AFT.Sqrt)
        rstd = spool.tile([CO, 1], dt.float32)
        nc.vector.reciprocal(rstd[:], std[:])
        sb = spool.tile([CO, 2], dt.float32)
        nc.vector.tensor_mul(sb[:, 0:1], rstd[:], gbt[:, 0:1])
        mus = spool.tile([CO, 1], dt.float32)
        nc.vector.tensor_mul(mus[:], mu, sb[:, 0:1])
        nc.vector.tensor_sub(sb[:, 1:2], gbt[:, 1:2], mus[:])
        # duplicate across both partition halves for 128-wide pass D
        sb2 = spool.tile([128, 2], dt.float32)
        nc.sync.dma_start(sb2[0:CO, :], sb[:])
        nc.sync.dma_start(sb2[CO:128, :], sb[:])

        # ---- Pass D: z=y*s+b; mish(z) = z*(q-1)/(q+1), q=(1+e^z)^2 ----
        for dd in range(TD // 2):
            yt = dpool.tile([128, OUTP], dt.float32)
            nc.sync.dma_start(yt[0:CO, :],
                              ydram[:, (2 * dd) * OUTP:(2 * dd + 1) * OUTP])
            nc.sync.dma_start(yt[CO:128, :],
                              ydram[:, (2 * dd + 1) * OUTP:(2 * dd + 2) * OUTP])
            e = dpool.tile([128, OUTP], dt.float32)
            nc.scalar.activation(e[:], yt[:], AFT.Exp,
                                 scale=sb2[:, 0:1], bias=sb2[:, 1:2])
            nc.scalar.activation(e[:], e[:], AFT.Square, bias=1.0)
            z = dpool.tile([128, OUTP], dt.float32)
            nc.scalar.activation(z[:], yt[:], AFT.Identity,
                                 scale=sb2[:, 0:1], bias=sb2[:, 1:2])
            r = dpool.tile([128, OUTP], dt.float32)
            nc.scalar.activation(r[:], e[:], AFT.Identity, bias=1.0)
            nc.vector.reciprocal_approx_fast(r[:], r[:])
            nc.vector.scalar_tensor_tensor(z[:], e[:], -1.0, z[:],
                                           mybir.AluOpType.add,
                                           mybir.AluOpType.mult)
            nc.vector.tensor_mul(z[:], z[:], r[:])
            nc.sync.dma_start(out_ap[:, (2 * dd) * OUTP:(2 * dd + 1) * OUTP],
                              z[0:CO, :])
            nc.sync.dma_start(out_ap[:, (2 * dd + 1) * OUTP:(2 * dd + 2) * OUTP],
                              z[CO:128, :])
    nc.compile()
    return nc


def _host_prep(x, t, w5, w3, w1, wa3, wa5, gate_w, gate_b, gamma, beta):
    f32 = np.float32
    x = np.asarray(x, f32)
    t = np.asarray(t, f32)
    logits = t @ np.asarray(gate_w, f32).T + np.asarray(gate_b, f32)
    lg = logits.reshape(N, E, CO)
    lg = lg - lg.max(axis=1, keepdims=True)
    eg = np.exp(lg)
    g = eg / eg.sum(axis=1, keepdims=True)          # [N, E, CO]

    def pad_k(w, p):
        return np.pad(np.asarray(w, f32),
                      ((0, 0), (0, 0), (p, p), (p, p), (p, p)))

    avg3 = np.full((3, 3, 3), 1.0 / 27.0, f32)
    avg5 = np.full((5, 5, 5), 1.0 / 125.0, f32)
    experts = np.stack([
        np.asarray(w5, f32),
        pad_k(w3, 1),
        pad_k(w1, 2),
        pad_k(np.asarray(wa3, f32) * avg3[None, None], 1),
        np.asarray(wa5, f32) * avg5[None, None],
    ])                                               # [E, CO, CI, 5,5,5]
    wmix = np.einsum('eoidhw,neo->noidhw', experts, g).astype(f32)

    wls = []
    for n in range(N):
        wm = wmix[n]                                 # [CO, CI, 5,5,5]
        wl = np.zeros((NCHUNK, 128, CO), f32)
        t1 = wm.transpose(2, 1, 3, 4, 0)             # [kd, ci, kh, kw, co]
        for c in range(25):
            kh, kw = divmod(c, 5)
            wl[c] = t1[0:4, :, kh, kw, :].reshape(128, CO)
        t2 = wm[:, :, 4, 0:4, :].transpose(2, 1, 3, 0)  # [kh(j), ci, kw, co]
        for kw in range(5):
            wl[25 + kw] = t2[:, :, kw, :].reshape(128, CO)
        t3 = wm[:, :, 4, 4, :].transpose(2, 1, 0)    # [kw, ci, co]
        wl[30] = t3[0:4].reshape(128, CO)            # kw 0..3 on row blocks
        wl[31][0:CI] = t3[4]                         # kw=4, K=32
        wls.append(np.ascontiguousarray(
            wl.transpose(1, 0, 2).reshape(128, NCHUNK * CO))
            .astype(ml_dtypes.bfloat16))

    gb = np.stack([np.asarray(gamma, f32), np.asarray(beta, f32)], axis=1)

    in_maps = []
    for c in range(NCORES):
        n, dh = divmod(c, 2)
        xpad = np.zeros((CI, NPLANES, PD, PD), f32)
        lo = dh * TD                # padded-plane base for this core
        # padded plane p (absolute) holds x depth p-2
        for p in range(NPLANES):
            src = lo + p - 2
            if 0 <= src < D:
                xpad[:, p, 2:2 + D, 2:2 + D] = x[n, :, src]
        in_maps.append({
            "xp": xpad.reshape(CI, NPLANES * PLANE).astype(ml_dtypes.bfloat16),
            "wl": wls[n],
            "gb": gb,
        })
    return in_maps


def kernel(x, t, w5, w3, w1, wa3, wa5, gate_w, gate_b, gamma, beta):
    from concourse.bass_utils import run_bass_kernel_spmd

    if "nc" not in _CACHE:
        _CACHE["nc"] = _build_nc()
    nc = _CACHE["nc"]

    in_maps = _host_prep(x, t, w5, w3, w1, wa3, wa5, gate_w, gate_b,
                         gamma, beta)
    res = run_bass_kernel_spmd(nc, in_maps, list(range(NCORES)))

    out = np.empty((N, CO, D, D, D), np.float32)
    for c in range(NCORES):
        n, dh = divmod(c, 2)
        out[n, :, dh * TD:(dh + 1) * TD] = \
            res.results[c]["out"].reshape(CO, TD, D, D)
    return out


# revision 27
# speedup vs baseline: 2.0310x; 1.0661x over previous
"""Trainium2 Bass kernel for nn_Net_12902081757308 (moe_routing).

Mixture-of-expert-kernels 3D conv + InstanceNorm + Mish.

Math: gate g = softmax_E(t @ gate_w.T + gate_b) [N,E,CO]; per-sample mixed
5^3 kernel w[n] = sum_e g[n,e,co] * expert_e[co,ci,kd,kh,kw]; y = conv3d(x, w)
SAME; InstanceNorm3d (biased var, eps=1e-5, affine gamma/beta); Mish.

Sharding (8 cores): core c -> (sample n=c//2, depth-half dh=c%2). Each core
computes all CO=64 channels for 24 of 48 output depth planes. InstanceNorm
stats are reduced across the core pairs with a tiny AllReduce.

Device algorithm per core:
  Pass B (conv): contraction (CI*5^3=4000) split into 35 matmul chunks:
    25 chunks (kh,kw): K=128 rows (kd 0..3 x ci) against a depth-replicated
      SBUF buffer A (partition block j = padded plane d+j),
    5 chunks (kw): K=128 rows (kh 0..3 x ci) at kd=4 against an h-shifted
      buffer C (partition block j = plane d+4 shifted j rows),
    5 leftover chunks (kw): K=32 rows (ci) at kd=4,kh=4 from C block 0.
  All 35 accumulate into PSUM [64co, 8h, 48w] in f32 (inputs f32r).
  ACT Copy/Square with accum_out collect per-channel sum/sumsq.
  Pass C: AllReduce partial stats over core pairs; build affine s,b with
    s=gamma*rstd, b=beta-mu*s.
  Pass D: mish(z)=z*(q-1)/(q+1), q=(1+e^z)^2, z=y*s+b via ACT Exp/Square
    + DVE rational, plane-pairs packed to 128 partitions.
"""
import ml_dtypes
import numpy as np
from contextlib import ExitStack

E, CI, CO, K, T = 5, 32, 64, 5, 3
N, D = 4, 48
PD = D + 4            # padded spatial extent (52)
PLANE = PD * PD       # 2704
TD = D // 2           # output depths per core (24)
NPLANES = TD + 5      # input planes staged per core (28 + 1 guard)
VOL = D * D * D       # 110592 elements per (n, co) instance
OUTP = D * D          # 2304 per output plane
EPS = 1e-5
NCORES = 8
NCHUNK = 32
HTILES = (0, 16, 32)       # col-pair tiles: (h0, h0+8) per PSUM tile
NTILES = TD * len(HTILES)  # 72 pair-tiles

_CACHE = {}


def _build_nc():
    from concourse import bacc, mybir, tile

    dt = mybir.dt
    AFT = mybir.ActivationFunctionType

    nc = bacc.Bacc("TRN2", target_bir_lowering=False, debug=False,
                   num_devices=NCORES)
    xp_ap = nc.dram_tensor("xp", [CI, NPLANES * PLANE], dt.bfloat16,
                           kind="ExternalInput").ap()
    wl_ap = nc.dram_tensor("wl", [128, NCHUNK * CO], dt.bfloat16,
                           kind="ExternalInput").ap()
    gb_ap = nc.dram_tensor("gb", [CO, 2], dt.float32,
                           kind="ExternalInput").ap()
    out_ap = nc.dram_tensor("out", [CO, TD * OUTP], dt.float32,
                            kind="ExternalOutput").ap()

    with tile.TileContext(nc) as tc, ExitStack() as ctx:
        cpool = ctx.enter_context(tc.tile_pool(name="const", bufs=1))
        spool = ctx.enter_context(tc.tile_pool(name="stats", bufs=1))
        drampool = ctx.enter_context(tc.tile_pool(name="dram", bufs=1,
                                                  space="DRAM"))
        bctx = ExitStack()  # pass-B pools, closed before pass D
        apool = bctx.enter_context(tc.tile_pool(name="abuf", bufs=3))
        cbpool = bctx.enter_context(tc.tile_pool(name="cbuf", bufs=3))
        wpool = bctx.enter_context(tc.tile_pool(name="wbuf", bufs=3))
        ppool = bctx.enter_context(tc.tile_pool(name="psum", bufs=8,
                                                space="PSUM"))
        ypool = bctx.enter_context(tc.tile_pool(name="ysb", bufs=4))

        wsb = cpool.tile([128, NCHUNK * CO], dt.bfloat16)
        nc.sync.dma_start(wsb[:], wl_ap[:])
        gbt = cpool.tile([CO, 2], dt.float32)
        nc.sync.dma_start(gbt[:], gb_ap[:])

        sums = spool.tile([128, NTILES], dt.float32)
        sumsq = spool.tile([128, NTILES], dt.float32)
        ydram = drampool.tile([CO, TD * OUTP], dt.float32)

        # ---- Pass B: conv + partial stats ----
        # rhs = strided [8 rows x 48 @ stride 52] windows of the flat padded
        # plane (bf16 sustains 1 col/cycle on strided reads). Two h-tiles
        # (h0, h0+8) run concurrently in the two PE column groups
        # (tile_position 0/64), doubling throughput for M=64.
        ti = 0
        for d in range(TD):
            A = apool.tile([128, PD, PD], dt.bfloat16)
            for j in range(4):
                nc.sync.dma_start(
                    A[32 * j:32 * j + 32],
                    xp_ap[:, (d + j) * PLANE:(d + j + 1) * PLANE])
            Ct = cbpool.tile([128, PD, PD], dt.bfloat16)
            for j in range(4):
                off = (d + 4) * PLANE + j * PD
                nc.sync.dma_start(Ct[32 * j:32 * j + 32],
                                  xp_ap[:, off:off + PLANE])
            # W4: plane d+4 shifted by (4 rows + j cols); block j serves
            # the kd=4,kh=4,kw=j leftovers as one K=128 chunk.
            W4 = wpool.tile([128, PD, PD], dt.bfloat16)
            for j in range(4):
                off = (d + 4) * PLANE + 4 * PD + j
                nc.sync.dma_start(W4[32 * j:32 * j + 32],
                                  xp_ap[:, off:off + PLANE])
            for h0 in HTILES:
                ps = ppool.tile([128, 8, 48], dt.float32)

                def mm(c, buf, ro, co_, k, start, stop):
                    for g, pb in ((0, 0), (1, 64)):
                        r0 = ro + 8 * g
                        nc.tensor.matmul(
                            ps[pb:pb + CO],
                            wsb[0:k, c * CO:(c + 1) * CO],
                            buf[0:k, r0:r0 + 8, co_:co_ + 48],
                            start=start, stop=stop,
                            tile_position=(0, pb))

                c = 0
                for kh in range(K):
                    for kw in range(K):
                        mm(c, A, h0 + kh, kw, 128, c == 0, False)
                        c += 1
                for kw in range(K):
                    mm(c, Ct, h0, kw, 128, False, False)
                    c += 1
                mm(c, W4, h0, 0, 128, False, False)   # kw 0..3
                c += 1
                mm(c, W4, h0, 4, 32, False, True)     # kw=4
                c += 1
                ysb = ypool.tile([128, 8, 48], dt.float32)
                nc.scalar.activation(ysb[:], ps[:], AFT.Copy,
                                     accum_out=sums[:, ti:ti + 1])
                sqsb = ypool.tile([128, 8, 48], dt.float32)
                nc.scalar.activation(sqsb[:], ps[:], AFT.Square,
                                     accum_out=sumsq[:, ti:ti + 1])
                base = d * OUTP + h0 * 48
                nc.sync.dma_start(
                    ydram[:, base:base + 384],
                    ysb[0:CO].rearrange("p a b -> p (a b)"))
                nc.sync.dma_start(
                    ydram[:, base + 384:base + 768],
                    ysb[CO:128].rearrange("p a b -> p (a b)"))
                ti += 1

        # ---- Pass C: stats reduce + AllReduce + affine ----
        red = spool.tile([128, 2], dt.float32)
        nc.vector.reduce_sum(red[:, 0:1], sums[:], axis=mybir.AxisListType.X)
        nc.vector.reduce_sum(red[:, 1:2], sumsq[:], axis=mybir.AxisListType.X)
        redhi = spool.tile([CO, 2], dt.float32)
        nc.sync.dma_start(redhi[:], red[CO:128, :])
        ccin = spool.tile([CO, 2], dt.float32)
        nc.vector.tensor_add(ccin[:], red[0:CO, :], redhi[:])
        cin_d = drampool.tile([CO, 2], dt.float32)
        cout_d = drampool.tile([CO, 2], dt.float32)
        nc.sync.dma_start(cin_d[:], ccin[:])
        nc.gpsimd.collective_compute(
            "AllReduce", mybir.AluOpType.add,
            replica_groups=[[0, 1], [2, 3], [4, 5], [6, 7]],
            ins=[cin_d.opt()], outs=[cout_d.opt()])
        st = spool.tile([CO, 2], dt.float32)
        nc.sync.dma_start(st[:], cout_d[:])

        stv = spool.tile([CO, 2], dt.float32)
        nc.vector.tensor_scalar_mul(stv[:], st[:], 1.0 / VOL)  # [mu, m2]
        mu = stv[:, 0:1]
        musq = spool.tile([CO, 1], dt.float32)
        nc.vector.tensor_mul(musq[:], mu, mu)
        var = spool.tile([CO, 1], dt.float32)
        nc.vector.tensor_scalar(var[:], stv[:, 1:2], musq[:, 0:1], EPS,
                                mybir.AluOpType.subtract, mybir.AluOpType.add)
        std = spool.tile([CO, 1], dt.float32)
        nc.scalar.activation(std[:], var[:], AFT.Sqrt)
        rstd = spool.tile([CO, 1], dt.float32)
        nc.vector.reciprocal(rstd[:], std[:])
        sb = spool.tile([CO, 2], dt.float32)
        nc.vector.tensor_mul(sb[:, 0:1], rstd[:], gbt[:, 0:1])
        mus = spool.tile([CO, 1], dt.float32)
        nc.vector.tensor_mul(mus[:], mu, sb[:, 0:1])
        nc.vector.tensor_sub(sb[:, 1:2], gbt[:, 1:2], mus[:])
        # duplicate across both partition halves for 128-wide pass D
        sb2 = spool.tile([128, 2], dt.float32)
        nc.sync.dma_start(sb2[0:CO, :], sb[:])
        nc.sync.dma_start(sb2[CO:128, :], sb[:])

        bctx.close()  # free pass-B pools for pass D tiles

        # ---- Pass D: mish(z) = z * tanh(ln(1 + e^z)), z = y*s+b ----
        # ACT ops batched per LUT table (exp -> ln -> tanh) to amortize
        # the ~1.3us table-switch cost; DVE only computes z and the final
        # multiply.
        ytpool = ctx.enter_context(tc.tile_pool(name="yt", bufs=5))
        epool = ctx.enter_context(tc.tile_pool(name="et", bufs=6))
        zpool = ctx.enter_context(tc.tile_pool(name="zt", bufs=6))
        PB = 4
        NP = TD // 2
        for b0 in range(0, NP, PB):
            batch = list(range(b0, min(b0 + PB, NP)))
            yts, es, zs = {}, {}, {}
            for dd in batch:
                yt = ytpool.tile([128, OUTP], dt.float32)
                nc.sync.dma_start(
                    yt[0:CO, :],
                    ydram[:, (2 * dd) * OUTP:(2 * dd + 1) * OUTP])
                nc.sync.dma_start(
                    yt[CO:128, :],
                    ydram[:, (2 * dd + 1) * OUTP:(2 * dd + 2) * OUTP])
                yts[dd] = yt
            for dd in batch:
                e = epool.tile([128, OUTP], dt.float32)
                nc.scalar.activation(e[:], yts[dd][:], AFT.Exp,
                                     scale=sb2[:, 0:1], bias=sb2[:, 1:2])
                es[dd] = e
            for dd in batch:
                z = zpool.tile([128, OUTP], dt.float32)
                nc.vector.tensor_scalar(z[:], yts[dd][:], sb2[:, 0:1],
                                        sb2[:, 1:2], mybir.AluOpType.mult,
                                        mybir.AluOpType.add)
                zs[dd] = z
            for dd in batch:
                nc.scalar.activation(es[dd][:], es[dd][:], AFT.Ln, bias=1.0)
            for dd in batch:
                nc.scalar.activation(es[dd][:], es[dd][:], AFT.Tanh)
            for dd in batch:
                nc.vector.tensor_mul(zs[dd][:], zs[dd][:], es[dd][:])
                nc.sync.dma_start(
                    out_ap[:, (2 * dd) * OUTP:(2 * dd + 1) * OUTP],
                    zs[dd][0:CO, :])
                nc.sync.dma_start(
                    out_ap[:, (2 * dd + 1) * OUTP:(2 * dd + 2) * OUTP],
                    zs[dd][CO:128, :])
    nc.compile()
    return nc


def _host_prep(x, t, w5, w3, w1, wa3, wa5, gate_w, gate_b, gamma, beta):
    f32 = np.float32
    x = np.asarray(x, f32)
    t = np.asarray(t, f32)
    logits = t @ np.asarray(gate_w, f32).T + np.asarray(gate_b, f32)
    lg = logits.reshape(N, E, CO)
    lg = lg - lg.max(axis=1, keepdims=True)
    eg = np.exp(lg)
    g = eg / eg.sum(axis=1, keepdims=True)          # [N, E, CO]

    def pad_k(w, p):
        return np.pad(np.asarray(w, f32),
                      ((0, 0), (0, 0), (p, p), (p, p), (p, p)))

    avg3 = np.full((3, 3, 3), 1.0 / 27.0, f32)
    avg5 = np.full((5, 5, 5), 1.0 / 125.0, f32)
    experts = np.stack([
        np.asarray(w5, f32),
        pad_k(w3, 1),
        pad_k(w1, 2),
        pad_k(np.asarray(wa3, f32) * avg3[None, None], 1),
        np.asarray(wa5, f32) * avg5[None, None],
    ])                                               # [E, CO, CI, 5,5,5]
    wmix = np.einsum('eoidhw,neo->noidhw', experts, g).astype(f32)

    wls = []
    for n in range(N):
        wm = wmix[n]                                 # [CO, CI, 5,5,5]
        wl = np.zeros((NCHUNK, 128, CO), f32)
        t1 = wm.transpose(2, 1, 3, 4, 0)             # [kd, ci, kh, kw, co]
        for c in range(25):
            kh, kw = divmod(c, 5)
            wl[c] = t1[0:4, :, kh, kw, :].reshape(128, CO)
        t2 = wm[:, :, 4, 0:4, :].transpose(2, 1, 3, 0)  # [kh(j), ci, kw, co]
        for kw in range(5):
            wl[25 + kw] = t2[:, :, kw, :].reshape(128, CO)
        t3 = wm[:, :, 4, 4, :].transpose(2, 1, 0)    # [kw, ci, co]
        wl[30] = t3[0:4].reshape(128, CO)            # kw 0..3 on row blocks
        wl[31][0:CI] = t3[4]                         # kw=4, K=32
        wls.append(np.ascontiguousarray(
            wl.transpose(1, 0, 2).reshape(128, NCHUNK * CO))
            .astype(ml_dtypes.bfloat16))

    gb = np.stack([np.asarray(gamma, f32), np.asarray(beta, f32)], axis=1)

    in_maps = []
    for c in range(NCORES):
        n, dh = divmod(c, 2)
        xpad = np.zeros((CI, NPLANES, PD, PD), f32)
        lo = dh * TD                # padded-plane base for this core
        # padded plane p (absolute) holds x depth p-2
        for p in range(NPLANES):
            src = lo + p - 2
            if 0 <= src < D:
                xpad[:, p, 2:2 + D, 2:2 + D] = x[n, :, src]
        in_maps.append({
            "xp": xpad.reshape(CI, NPLANES * PLANE).astype(ml_dtypes.bfloat16),
            "wl": wls[n],
            "gb": gb,
        })
    return in_maps


def kernel(x, t, w5, w3, w1, wa3, wa5, gate_w, gate_b, gamma, beta):
    from concourse.bass_utils import run_bass_kernel_spmd

    if "nc" not in _CACHE:
        _CACHE["nc"] = _build_nc()
    nc = _CACHE["nc"]

    in_maps = _host_prep(x, t, w5, w3, w1, wa3, wa5, gate_w, gate_b,
                         gamma, beta)
    res = run_bass_kernel_spmd(nc, in_maps, list(range(NCORES)))

    out = np.empty((N, CO, D, D, D), np.float32)
    for c in range(NCORES):
        n, dh = divmod(c, 2)
        out[n, :, dh * TD:(dh + 1) * TD] = \
            res.results[c]["out"].reshape(CO, TD, D, D)
    return out
